# revision 16
# baseline (speedup 1.0000x reference)
"""Trainium2 Bass kernel for nn_BLCD_Loss (retrieval kNN hinge loss).

Math (reference):
  yin = l2norm(yi), yit = l2norm(yi_t)
  dis[i,j] = sqrt(max(|yin_i|^2+|yin_j|^2-2 yin_i.yin_j, 0) + 1e-12)
  top-(K+1) smallest per row (rank0 = self); neighbors = ranks 1..16
  e1 = sum relu((0.5*sqrt(|yin_i-yin_j|^2+eps) - 0.5*sqrt(|yit_i-yin_j|^2+eps))^2 - T)
  e2 = sum relu(0.5*sqrt(|yin_i-yit_i|^2+eps) + M - 0.5*sqrt(|yin_i-yij|^2+eps))

Kernel strategy (8 cores, SPMD) — single-scan packed-candidate scheme:
  Each core owns 1024 rows; host rotates yi so the self-match diagonal sits in
  local chunk 0.  Per 128-row tile and per 1024-column chunk:
    ps_t = yit_loc @ yinT          -> db_c = fp16(sqrt(...)*2^-10)  (ACT evict)
    ps_a = yin_loc @ yinT          -> s_q  = fp16(s)                (Pool/ACT evict)
    ps_c = I @ s_q + (I*2^-6) @ db_c   (two identity matmuls, f32 PSUM)
         = s_q + db*2^-16
  The DVE does ONE max8 scan per chunk over ps_c.  Because s_q is an exact
  fp16 value and db*2^-16 < ulp_fp16(s_q)/2 for any candidate-sized s, an
  fp16 round-trip of a candidate value recovers s_q exactly and the residual
  times 2^16 recovers db to ~2^-9 — enough for the hinge (validated offline:
  rel err 1.2e-3 vs reference on the fixed dataset, incl. bf16 matmul inputs).
  The self column is rank0 of chunk 0 (s=1); dis(yin_i,yit_i) for e2 comes
  from the chunk-0 diagonal of db_c.  Top-16 threshold/mask + hinge run on
  [128, 64] candidate tiles; no full-row elementwise work exists anywhere:
  the only full-width ops are 4 matmul passes (PE), one sqrt eviction (ACT),
  one fp16 copy (Pool/ACT) and one max8 scan (DVE) per chunk.
"""

import os
import numpy as np

N, D = 8192, 128
NCORES = 8
ROWS = N // NCORES          # 1024 rows per core
NRT = ROWS // 128           # 8 row-tiles per core
NT = N // 128               # 64 column tiles
CH = 1024                   # PSUM chunk width (2 banks)
NCH = N // CH               # 8 chunks per row-tile
T_THR = 0.0025
MARGIN = 0.5
EPS = 1e-12
C0 = 0.5 + 0.25e-12         # dis = sqrt(s*(-0.5) + C0)
KS = 1024.0                 # db_c = db/KS (fp16-normal range)
PK = 64.0                   # eye scale KS*PK = 2^16 total packing
NEG = -1.0e30               # match_replace fill
KNOCK = -60000.0            # fp16-safe knock value

_CACHE = {}


def _build_module():
    import concourse.bass as bass  # noqa: F401
    import concourse.tile as tile
    from contextlib import ExitStack
    from concourse import bacc, mybir

    CFG = os.environ.get("BLCD_CFG", "")
    def knob(name, default):
        for part in CFG.split(","):
            if part.startswith(name + "="):
                return int(part.split("=")[1])
        return default
    SQD = knob("sqd", 3)     # s_q-evict chunks on DVE (from the end; rest ACT)

    f32 = mybir.dt.float32
    bf16 = mybir.dt.bfloat16
    fp16 = mybir.dt.float16
    AF = mybir.ActivationFunctionType
    ALU = mybir.AluOpType
    AX = mybir.AxisListType

    nc = bacc.Bacc("TRN2", target_bir_lowering=False, debug=False,
                   num_devices=NCORES)

    yi_d = nc.dram_tensor("yi_rot", [N, D], f32, kind="ExternalInput")
    yit_d = nc.dram_tensor("yit_loc", [ROWS, D], f32, kind="ExternalInput")
    eye_d = nc.dram_tensor("eye1", [128, 128], f32, kind="ExternalInput")
    out_d = nc.dram_tensor("out", [128, 2], f32, kind="ExternalOutput")

    yi_r = yi_d.ap().rearrange("(n p) d -> p n d", p=128)     # [128, 64, 128]
    yit_r = yit_d.ap().rearrange("(n p) d -> p n d", p=128)   # [128, 8, 128]

    with tile.TileContext(nc) as tc, ExitStack() as ctx:
        cpool = ctx.enter_context(tc.tile_pool(name="consts", bufs=1))
        ppool = ctx.enter_context(tc.tile_pool(name="persist", bufs=1))
        smpool = ctx.enter_context(tc.tile_pool(name="small", bufs=2))

        eye = cpool.tile([128, 128], f32)
        nc.sync.dma_start(eye[:], eye_d[:])
        eyeh = cpool.tile([128, 128], fp16)
        nc.gpsimd.tensor_copy(eyeh[:], eye[:])
        eyep = cpool.tile([128, 128], fp16)
        nc.gpsimd.tensor_scalar(eyep[:], eyeh[:], 1.0 / PK, None, ALU.mult)
        c0b = cpool.tile([128, 1], f32)
        nc.gpsimd.memset(c0b[:], C0)
        c0s = cpool.tile([128, 1], f32)
        nc.gpsimd.memset(c0s[:], C0 / (KS * KS))
        epsb = cpool.tile([128, 1], f32)
        nc.gpsimd.memset(epsb[:], EPS)

        yinT = ppool.tile([128, N], bf16)        # normalized yi, transposed
        yitT = ppool.tile([128, ROWS], bf16)     # normalized yi_t, transposed
        e1acc = ppool.tile([128, NRT], f32)
        e2acc = ppool.tile([128, NRT], f32)

        # ---------------- head: normalize + transpose ----------------
        # processed in 8-block groups so early yinT columns unblock the
        # main-loop matmuls long before the whole head finishes
        with tc.tile_pool(name="headbig", bufs=4) as hbig, \
             tc.tile_pool(name="headsm", bufs=4) as hsm, \
             tc.tile_pool(name="headps", bufs=4, space="PSUM") as hpsum:
            order = [(yi_r, 0, yinT), (yit_r, 0, yitT)] + \
                    [(yi_r, g, yinT) for g in range(8, NT, 8)]
            for (src_r, g, dstT) in order:
                rows = hbig.tile([128, 8, 128], f32, tag="rows")
                nc.sync.dma_start(rows[:], src_r[:, g:g + 8, :])
                sqr = hbig.tile([128, 128], f32, tag="sqr")
                sq = hsm.tile([128, 8], f32, tag="sq")
                for jj in range(8):
                    # row norms via Square+accum: keeps the head off the DVE
                    nc.scalar.activation(sqr[:], rows[:, jj, :], AF.Square,
                                         accum_out=sq[:, jj:jj + 1])
                nrm = hsm.tile([128, 8], f32, tag="nrm")
                nc.scalar.activation(nrm[:], sq[:], AF.Sqrt, bias=epsb[:])
                rinv = hsm.tile([128, 8], f32, tag="rinv")
                nc.vector.reciprocal(rinv[:], nrm[:])
                for jj in range(8):
                    j = g + jj
                    # diag(rinv) built on Pool; PE matmul y.T @ diag(r)
                    # fuses the normalize scaling into the transpose
                    diagm = hsm.tile([128, 128], f32, tag="diagm")
                    nc.gpsimd.tensor_scalar(diagm[:], eye[:],
                                            rinv[:, jj:jj + 1], None,
                                            ALU.mult)
                    ps = hpsum.tile([128, 128], f32, tag="tps")
                    nc.tensor.matmul(ps[:], rows[:, jj, :], diagm[:],
                                     start=True, stop=True)
                    nc.scalar.copy(dstT[:, j * 128:(j + 1) * 128], ps[:])

        # ---------------- main loop over 8 row-tiles ----------------
        with tc.tile_pool(name="dbp", bufs=2) as dbpool, \
             tc.tile_pool(name="sqp", bufs=2) as sqpool, \
             tc.tile_pool(name="ps_t", bufs=1, space="PSUM") as ps_tpool, \
             tc.tile_pool(name="ps_a", bufs=2, space="PSUM") as ps_apool, \
             tc.tile_pool(name="ps_c", bufs=1, space="PSUM") as ps_cpool:
            for rt in range(NRT):
                lhs_s = yinT[:, rt * 128:(rt + 1) * 128]
                lhs_t = yitT[:, rt * 128:(rt + 1) * 128]
                cand = smpool.tile([128, NCH * 8], f32, tag="cand")
                dis_td = smpool.tile([128, 1], f32, tag="dtd")
                for cc in range(NCH):
                    ps_t = ps_tpool.tile([128, CH], f32)
                    for h in range(2):
                        rhs = yinT[:, cc * CH + h * 512: cc * CH + (h + 1) * 512]
                        nc.tensor.matmul(ps_t[:, h * 512:(h + 1) * 512],
                                         lhs_t, rhs, start=True, stop=True)
                    # db scaled by 1/KS: sqrt((-0.5 t + C0)/KS^2), fp16
                    db_c = dbpool.tile([128, CH], fp16, tag="db")
                    nc.scalar.activation(db_c[:], ps_t[:], AF.Sqrt,
                                         scale=-0.5 / (KS * KS), bias=c0s[:])
                    if cc == 0:
                        # e2: dis(yin_i, yit_i)*(1/KS) on the chunk-0 diagonal
                        dsl = slice(rt * 128, (rt + 1) * 128)
                        tdscr = smpool.tile([128, 128], fp16, tag="tdscr")
                        nc.gpsimd.tensor_tensor(tdscr[:], db_c[:, dsl],
                                                eyeh[:], op=ALU.mult)
                        tdr = smpool.tile([128, 1], f32, tag="tdr")
                        nc.vector.tensor_reduce(tdr[:], tdscr[:], op=ALU.add,
                                                axis=AX.X)
                        nc.gpsimd.tensor_scalar(dis_td[:], tdr[:], KS, None,
                                                ALU.mult)
                    ps_a = ps_apool.tile([128, CH], f32)
                    for h in range(2):
                        rhs = yinT[:, cc * CH + h * 512: cc * CH + (h + 1) * 512]
                        nc.tensor.matmul(ps_a[:, h * 512:(h + 1) * 512],
                                         lhs_s, rhs, start=True, stop=True)
                    s_q = sqpool.tile([128, CH], fp16, tag="sq")
                    if cc >= NCH - SQD:
                        nc.vector.tensor_copy(s_q[:], ps_a[:])
                    else:
                        nc.scalar.copy(s_q[:], ps_a[:])
                    ps_c = ps_cpool.tile([128, CH], f32)
                    for h in range(2):
                        hs = slice(h * 512, (h + 1) * 512)
                        nc.tensor.matmul(ps_c[:, hs], eyeh[:], s_q[:, hs],
                                         start=True, stop=False)
                        nc.tensor.matmul(ps_c[:, hs], eyep[:], db_c[:, hs],
                                         start=False, stop=True)
                    nc.vector.max(cand[:, cc * 8:(cc + 1) * 8], ps_c[:])

                # ---- candidate-space math ([128, 64] tiles) ----
                # knock self (rank0 of chunk0; s=1 dominates)
                nc.gpsimd.memset(cand[:, 0:1], KNOCK)

                # theta = 16th largest candidate
                r1 = smpool.tile([128, 8], f32, tag="r1")
                r2 = smpool.tile([128, 8], f32, tag="r2")
                cbk = smpool.tile([128, NCH * 8], f32, tag="cbk")
                nc.vector.max(r1[:], cand[:])
                nc.vector.match_replace(cbk[:], r1[:], cand[:], NEG)
                nc.vector.max(r2[:], cbk[:])

                # decode: s_q = fp16 round-trip, db = (C - s_q)*2^16
                cq = smpool.tile([128, NCH * 8], fp16, tag="cq")
                nc.scalar.copy(cq[:], cand[:])
                d64 = smpool.tile([128, NCH * 8], f32, tag="d64")
                nc.gpsimd.tensor_tensor(d64[:], cand[:], cq[:],
                                        op=ALU.subtract)
                db64 = smpool.tile([128, NCH * 8], f32, tag="db64")
                nc.gpsimd.tensor_scalar(db64[:], d64[:], KS * PK, None,
                                        ALU.mult)
                da64 = smpool.tile([128, NCH * 8], f32, tag="da64")
                nc.scalar.activation(da64[:], cq[:], AF.Sqrt,
                                     scale=-0.5, bias=c0b[:])
                mk64 = smpool.tile([128, NCH * 8], f32, tag="mk64")
                nc.vector.tensor_scalar(mk64[:], cand[:], r2[:, 7:8], None,
                                        ALU.is_ge)
                w = smpool.tile([128, NCH * 8], f32, tag="w")
                nc.vector.tensor_tensor(w[:], da64[:], db64[:],
                                        op=ALU.subtract)
                nc.vector.tensor_tensor(w[:], w[:], mk64[:], op=ALU.mult)
                nc.vector.tensor_tensor(w[:], w[:], w[:], op=ALU.mult)
                # accum_out reduces with op1 -> relu (max) and summing
                # accumulate (add) stay separate instructions
                nc.vector.tensor_scalar(w[:], w[:], T_THR, 0.0,
                                        ALU.subtract, ALU.max)
                nc.vector.tensor_scalar(w[:], w[:], 1.0, None,
                                        ALU.mult, ALU.add,
                                        accum_out=e1acc[:, rt:rt + 1])

                # e2 row terms: nearest neighbor (rank-1 after self-knock)
                r1a = smpool.tile([128, 8], f32, tag="r1a")
                nc.vector.max(r1a[:], cand[:])
                r1aq = smpool.tile([128, 1], fp16, tag="r1aq")
                nc.scalar.copy(r1aq[:], r1a[:, 0:1])
                dis_nn = smpool.tile([128, 1], f32, tag="dnn")
                nc.scalar.activation(dis_nn[:], r1aq[:], AF.Sqrt,
                                     scale=-0.5, bias=c0b[:])
                o2 = smpool.tile([128, 1], f32, tag="o2")
                nc.vector.tensor_scalar(o2[:], dis_td[:], dis_nn[:, 0:1],
                                        MARGIN, ALU.subtract, ALU.add)
                nc.vector.tensor_scalar(e2acc[:, rt:rt + 1], o2[:], 0.0, None,
                                        ALU.max)

        # ---------------- tail: reduce + store ----------------
        e1r = smpool.tile([128, 1], f32, tag="e1r")
        e2r = smpool.tile([128, 1], f32, tag="e2r")
        nc.vector.tensor_reduce(e1r[:], e1acc[:], op=ALU.add, axis=AX.X)
        nc.vector.tensor_reduce(e2r[:], e2acc[:], op=ALU.add, axis=AX.X)
        nc.sync.dma_start(out_d[:, 0:1], e1r[:])
        nc.sync.dma_start(out_d[:, 1:2], e2r[:])

    nc.compile()
    return nc


def kernel(yi: np.ndarray, yi_t: np.ndarray):
    from concourse.bass_utils import run_bass_kernel_spmd

    if "nc" not in _CACHE:
        _CACHE["nc"] = _build_module()
    nc = _CACHE["nc"]

    yi = np.ascontiguousarray(np.asarray(yi, dtype=np.float32))
    yi_t = np.ascontiguousarray(np.asarray(yi_t, dtype=np.float32))
    eye1 = np.eye(128, dtype=np.float32)

    in_maps = []
    for c in range(NCORES):
        lo = c * ROWS
        yi_rot = np.concatenate([yi[lo:], yi[:lo]], axis=0)
        in_maps.append({
            "yi_rot": np.ascontiguousarray(yi_rot),
            "yit_loc": np.ascontiguousarray(yi_t[lo:lo + ROWS]),
            "eye1": eye1,
        })

    res = run_bass_kernel_spmd(nc, in_maps, list(range(NCORES))).results

    e1 = np.float64(0.0)
    e2 = np.float64(0.0)
    for c in range(NCORES):
        out = res[c]["out"]
        e1 += out[:, 0].astype(np.float64).sum()
        e2 += out[:, 1].astype(np.float64).sum()
    e1 = np.float32(e1)
    e2 = np.float32(e2)
    return (np.float32(e1 + e2), e1, e2)


# revision 17
# speedup vs baseline: 1.1114x; 1.1114x over previous
"""Trainium2 Bass kernel for nn_BLCD_Loss (retrieval kNN hinge loss).

Math (reference):
  yin = l2norm(yi), yit = l2norm(yi_t)
  dis[i,j] = sqrt(max(|yin_i|^2+|yin_j|^2-2 yin_i.yin_j, 0) + 1e-12)
  top-(K+1) smallest per row (rank0 = self); neighbors = ranks 1..16
  e1 = sum relu((0.5*sqrt(|yin_i-yin_j|^2+eps) - 0.5*sqrt(|yit_i-yin_j|^2+eps))^2 - T)
  e2 = sum relu(0.5*sqrt(|yin_i-yit_i|^2+eps) + M - 0.5*sqrt(|yin_i-yij|^2+eps))

Kernel strategy (8 cores, SPMD) — single-scan packed-candidate scheme:
  Each core owns 1024 rows; host rotates yi so the self-match diagonal sits in
  the local diagonal block.  Per 128-row tile and per 512-column chunk:
    ps_t = yit_loc @ yinT       -> db_c = sqrt(...)*2^-16   (ACT evict, f32)
    ps_a = yin_loc @ yinT       -> s_q  = fp16(s)           (ACT/DVE evict)
    ps_c = I_fp16 @ s_q + I_f32r @ db_c   (identity matmuls, f32 PSUM)
         = s_q + db*2^-16
  The DVE does ONE max8 scan per chunk over ps_c.  Because s_q is an exact
  fp16 value and db*2^-16 < ulp_fp16(s_q)/2 for any candidate-sized s, an
  fp16 round-trip of a candidate value recovers s_q exactly and the residual
  times 2^16 recovers db to ~2^-9 — enough for the hinge (validated offline:
  rel err ~1.2e-3 vs reference on the fixed dataset).  The self column is
  rank0 of its chunk (s=1 dominates); dis(yin_i,yit_i) for e2 comes from the
  diagonal block of db_c.  Top-16 threshold/mask + hinge run on [128, 128]
  candidate tiles.  No full-row elementwise work exists anywhere: per chunk
  the full-width ops are 4 matmuls (PE), one sqrt eviction (ACT), one fp16
  copy (ACT/DVE split) and one max8 scan (DVE).
"""

import os
import numpy as np

N, D = 8192, 128
NCORES = 8
ROWS = N // NCORES          # 1024 rows per core
NRT = ROWS // 128           # 8 row-tiles per core
NT = N // 128               # 64 column tiles
CH = 512                    # PSUM chunk width (1 bank)
NCH = N // CH               # 16 chunks per row-tile
T_THR = 0.0025
MARGIN = 0.5
EPS = 1e-12
C0 = 0.5 + 0.25e-12         # dis = sqrt(s*(-0.5) + C0)
PK = 65536.0                # db packing scale: C = s_q + db/PK
NEG = -1.0e30               # match_replace fill
KNOCK = -60000.0            # fp16-safe knock value

_CACHE = {}


def _build_module():
    import concourse.bass as bass  # noqa: F401
    import concourse.tile as tile
    from contextlib import ExitStack
    from concourse import bacc, mybir

    CFG = os.environ.get("BLCD_CFG", "")
    def knob(name, default):
        for part in CFG.split(","):
            if part.startswith(name + "="):
                return int(part.split("=")[1])
        return default
    SQD = knob("sqd", 6)     # s_q-evict chunks on DVE (of 16; rest ACT)
    PST = knob("pst", 2)     # PSUM bufs for t
    PSA = knob("psa", 2)     # PSUM bufs for s
    PSC = knob("psc", 3)     # PSUM bufs for packed C

    f32 = mybir.dt.float32
    f32r = mybir.dt.float32r
    fp16 = mybir.dt.float16
    AF = mybir.ActivationFunctionType
    ALU = mybir.AluOpType
    AX = mybir.AxisListType

    nc = bacc.Bacc("TRN2", target_bir_lowering=False, debug=False,
                   num_devices=NCORES)

    yi_d = nc.dram_tensor("yi_rot", [N, D], f32, kind="ExternalInput")
    yit_d = nc.dram_tensor("yit_loc", [ROWS, D], f32, kind="ExternalInput")
    eye_d = nc.dram_tensor("eye1", [128, 128], f32, kind="ExternalInput")
    out_d = nc.dram_tensor("out", [128, 2], f32, kind="ExternalOutput")

    yi_r = yi_d.ap().rearrange("(n p) d -> p n d", p=128)     # [128, 64, 128]
    yit_r = yit_d.ap().rearrange("(n p) d -> p n d", p=128)   # [128, 8, 128]

    with tile.TileContext(nc) as tc, ExitStack() as ctx:
        cpool = ctx.enter_context(tc.tile_pool(name="consts", bufs=1))
        ppool = ctx.enter_context(tc.tile_pool(name="persist", bufs=1))
        smpool = ctx.enter_context(tc.tile_pool(name="small", bufs=2))

        eye = cpool.tile([128, 128], f32)
        nc.sync.dma_start(eye[:], eye_d[:])
        eyer = cpool.tile([128, 128], f32r)
        nc.gpsimd.tensor_copy(eyer[:], eye[:])
        eyeh = cpool.tile([128, 128], fp16)
        nc.gpsimd.tensor_copy(eyeh[:], eye[:])
        c0b = cpool.tile([128, 1], f32)
        nc.gpsimd.memset(c0b[:], C0)
        c0s = cpool.tile([128, 1], f32)
        nc.gpsimd.memset(c0s[:], C0 / (PK * PK))
        epsb = cpool.tile([128, 1], f32)
        nc.gpsimd.memset(epsb[:], EPS)

        yinT = ppool.tile([128, N], f32r)        # normalized yi, transposed
        yitT = ppool.tile([128, ROWS], f32r)     # normalized yi_t, transposed
        e1acc = ppool.tile([128, NRT], f32)
        e2acc = ppool.tile([128, NRT], f32)

        # ---------------- head: normalize + transpose ----------------
        # processed in 8-block groups so early yinT columns unblock the
        # main-loop matmuls long before the whole head finishes
        with tc.tile_pool(name="headbig", bufs=4) as hbig, \
             tc.tile_pool(name="headsm", bufs=4) as hsm, \
             tc.tile_pool(name="headps", bufs=4, space="PSUM") as hpsum:
            order = [(yi_r, 0, yinT), (yit_r, 0, yitT)] + \
                    [(yi_r, g, yinT) for g in range(8, NT, 8)]
            for (src_r, g, dstT) in order:
                rows = hbig.tile([128, 8, 128], f32, tag="rows")
                nc.sync.dma_start(rows[:], src_r[:, g:g + 8, :])
                sqr = hbig.tile([128, 128], f32, tag="sqr")
                sq = hsm.tile([128, 8], f32, tag="sq")
                for jj in range(8):
                    # row norms via Square+accum: keeps the head off the DVE
                    nc.scalar.activation(sqr[:], rows[:, jj, :], AF.Square,
                                         accum_out=sq[:, jj:jj + 1])
                nrm = hsm.tile([128, 8], f32, tag="nrm")
                nc.scalar.activation(nrm[:], sq[:], AF.Sqrt, bias=epsb[:])
                rinv = hsm.tile([128, 8], f32, tag="rinv")
                nc.vector.reciprocal(rinv[:], nrm[:])
                for jj in range(8):
                    j = g + jj
                    # diag(rinv) built on Pool; PE matmul y.T @ diag(r)
                    # fuses the normalize scaling into the transpose
                    diagm = hsm.tile([128, 128], f32, tag="diagm")
                    nc.gpsimd.tensor_scalar(diagm[:], eye[:],
                                            rinv[:, jj:jj + 1], None,
                                            ALU.mult)
                    ps = hpsum.tile([128, 128], f32, tag="tps")
                    nc.tensor.matmul(ps[:], rows[:, jj, :], diagm[:],
                                     start=True, stop=True)
                    nc.scalar.copy(dstT[:, j * 128:(j + 1) * 128], ps[:])

        # ---------------- main loop over 8 row-tiles ----------------
        with tc.tile_pool(name="dbp", bufs=3) as dbpool, \
             tc.tile_pool(name="sqp", bufs=3) as sqpool, \
             tc.tile_pool(name="ps_t", bufs=PST, space="PSUM") as ps_tpool, \
             tc.tile_pool(name="ps_a", bufs=PSA, space="PSUM") as ps_apool, \
             tc.tile_pool(name="ps_c", bufs=PSC, space="PSUM") as ps_cpool:
            for rt in range(NRT):
                lhs_s = yinT[:, rt * 128:(rt + 1) * 128]
                lhs_t = yitT[:, rt * 128:(rt + 1) * 128]
                cand = smpool.tile([128, NCH * 8], f32, tag="cand")
                dis_td = smpool.tile([128, 1], f32, tag="dtd")
                c_self = (rt * 128) // CH
                o_self = (rt * 128) % CH
                for cc in range(NCH):
                    rhs = yinT[:, cc * CH:(cc + 1) * CH]
                    ps_t = ps_tpool.tile([128, CH], f32)
                    nc.tensor.matmul(ps_t[:], lhs_t, rhs, start=True,
                                     stop=True)
                    # db pre-scaled by 1/PK: sqrt((-0.5 t + C0)/PK^2), f32
                    db_c = dbpool.tile([128, CH], f32r, tag="db")
                    nc.scalar.activation(db_c[:], ps_t[:], AF.Sqrt,
                                         scale=-0.5 / (PK * PK), bias=c0s[:])
                    if cc == c_self:
                        # e2: dis(yin_i, yit_i)/PK on the diagonal block
                        dsl = slice(o_self, o_self + 128)
                        tdscr = smpool.tile([128, 128], f32, tag="tdscr")
                        nc.gpsimd.tensor_tensor(tdscr[:], db_c[:, dsl],
                                                eye[:], op=ALU.mult)
                        tdr = smpool.tile([128, 1], f32, tag="tdr")
                        nc.vector.tensor_reduce(tdr[:], tdscr[:], op=ALU.add,
                                                axis=AX.X)
                        nc.gpsimd.tensor_scalar(dis_td[:], tdr[:], PK, None,
                                                ALU.mult)
                    ps_a = ps_apool.tile([128, CH], f32)
                    nc.tensor.matmul(ps_a[:], lhs_s, rhs, start=True,
                                     stop=True)
                    s_q = sqpool.tile([128, CH], fp16, tag="sq")
                    if cc % 8 >= 8 - SQD // 2:
                        nc.vector.tensor_copy(s_q[:], ps_a[:])
                    else:
                        nc.scalar.copy(s_q[:], ps_a[:])
                    ps_c = ps_cpool.tile([128, CH], f32)
                    nc.tensor.matmul(ps_c[:], eyeh[:], s_q[:],
                                     start=True, stop=False)
                    nc.tensor.matmul(ps_c[:], eyer[:], db_c[:],
                                     start=False, stop=True)
                    nc.vector.max(cand[:, cc * 8:(cc + 1) * 8], ps_c[:])

                # ---- candidate-space math ([128, 128] tiles) ----
                # knock self (rank0 of its chunk; s=1 dominates)
                nc.gpsimd.memset(cand[:, c_self * 8:c_self * 8 + 1], KNOCK)

                # theta = 16th largest candidate
                r1 = smpool.tile([128, 8], f32, tag="r1")
                r2 = smpool.tile([128, 8], f32, tag="r2")
                cbk = smpool.tile([128, NCH * 8], f32, tag="cbk")
                nc.vector.max(r1[:], cand[:])
                nc.vector.match_replace(cbk[:], r1[:], cand[:], NEG)
                nc.vector.max(r2[:], cbk[:])

                # decode: s_q = fp16 round-trip, db = (C - s_q)*PK
                cq = smpool.tile([128, NCH * 8], fp16, tag="cq")
                nc.scalar.copy(cq[:], cand[:])
                d64 = smpool.tile([128, NCH * 8], f32, tag="d64")
                nc.gpsimd.tensor_tensor(d64[:], cand[:], cq[:],
                                        op=ALU.subtract)
                db64 = smpool.tile([128, NCH * 8], f32, tag="db64")
                nc.gpsimd.tensor_scalar(db64[:], d64[:], PK, None,
                                        ALU.mult)
                da64 = smpool.tile([128, NCH * 8], f32, tag="da64")
                nc.scalar.activation(da64[:], cq[:], AF.Sqrt,
                                     scale=-0.5, bias=c0b[:])
                mk64 = smpool.tile([128, NCH * 8], f32, tag="mk64")
                nc.vector.tensor_scalar(mk64[:], cand[:], r2[:, 7:8], None,
                                        ALU.is_ge)
                w = smpool.tile([128, NCH * 8], f32, tag="w")
                nc.vector.tensor_tensor(w[:], da64[:], db64[:],
                                        op=ALU.subtract)
                nc.vector.tensor_tensor(w[:], w[:], mk64[:], op=ALU.mult)
                nc.vector.tensor_tensor(w[:], w[:], w[:], op=ALU.mult)
                # accum_out reduces with op1 -> relu (max) and summing
                # accumulate (add) stay separate instructions
                nc.vector.tensor_scalar(w[:], w[:], T_THR, 0.0,
                                        ALU.subtract, ALU.max)
                nc.vector.tensor_scalar(w[:], w[:], 1.0, None,
                                        ALU.mult, ALU.add,
                                        accum_out=e1acc[:, rt:rt + 1])

                # e2 row terms: nearest neighbor (rank-1 after self-knock)
                r1a = smpool.tile([128, 8], f32, tag="r1a")
                nc.vector.max(r1a[:], cand[:])
                r1aq = smpool.tile([128, 1], fp16, tag="r1aq")
                nc.scalar.copy(r1aq[:], r1a[:, 0:1])
                dis_nn = smpool.tile([128, 1], f32, tag="dnn")
                nc.scalar.activation(dis_nn[:], r1aq[:], AF.Sqrt,
                                     scale=-0.5, bias=c0b[:])
                o2 = smpool.tile([128, 1], f32, tag="o2")
                nc.vector.tensor_scalar(o2[:], dis_td[:], dis_nn[:, 0:1],
                                        MARGIN, ALU.subtract, ALU.add)
                nc.vector.tensor_scalar(e2acc[:, rt:rt + 1], o2[:], 0.0, None,
                                        ALU.max)

        # ---------------- tail: reduce + store ----------------
        e1r = smpool.tile([128, 1], f32, tag="e1r")
        e2r = smpool.tile([128, 1], f32, tag="e2r")
        nc.vector.tensor_reduce(e1r[:], e1acc[:], op=ALU.add, axis=AX.X)
        nc.vector.tensor_reduce(e2r[:], e2acc[:], op=ALU.add, axis=AX.X)
        nc.sync.dma_start(out_d[:, 0:1], e1r[:])
        nc.sync.dma_start(out_d[:, 1:2], e2r[:])

    nc.compile()
    return nc


def kernel(yi: np.ndarray, yi_t: np.ndarray):
    from concourse.bass_utils import run_bass_kernel_spmd

    if "nc" not in _CACHE:
        _CACHE["nc"] = _build_module()
    nc = _CACHE["nc"]

    yi = np.ascontiguousarray(np.asarray(yi, dtype=np.float32))
    yi_t = np.ascontiguousarray(np.asarray(yi_t, dtype=np.float32))
    eye1 = np.eye(128, dtype=np.float32)

    in_maps = []
    for c in range(NCORES):
        lo = c * ROWS
        yi_rot = np.concatenate([yi[lo:], yi[:lo]], axis=0)
        in_maps.append({
            "yi_rot": np.ascontiguousarray(yi_rot),
            "yit_loc": np.ascontiguousarray(yi_t[lo:lo + ROWS]),
            "eye1": eye1,
        })

    res = run_bass_kernel_spmd(nc, in_maps, list(range(NCORES))).results

    e1 = np.float64(0.0)
    e2 = np.float64(0.0)
    for c in range(NCORES):
        out = res[c]["out"]
        e1 += out[:, 0].astype(np.float64).sum()
        e2 += out[:, 1].astype(np.float64).sum()
    e1 = np.float32(e1)
    e2 = np.float32(e2)
    return (np.float32(e1 + e2), e1, e2)


# revision 20
# speedup vs baseline: 1.1229x; 1.0104x over previous
"""Trainium2 Bass kernel for nn_BLCD_Loss (retrieval kNN hinge loss).

Math (reference):
  yin = l2norm(yi), yit = l2norm(yi_t)
  dis[i,j] = sqrt(max(|yin_i|^2+|yin_j|^2-2 yin_i.yin_j, 0) + 1e-12)
  top-(K+1) smallest per row (rank0 = self); neighbors = ranks 1..16
  e1 = sum relu((0.5*sqrt(|yin_i-yin_j|^2+eps) - 0.5*sqrt(|yit_i-yin_j|^2+eps))^2 - T)
  e2 = sum relu(0.5*sqrt(|yin_i-yit_i|^2+eps) + M - 0.5*sqrt(|yin_i-yij|^2+eps))

Kernel strategy (8 cores, SPMD) — single-scan packed-candidate scheme:
  Each core owns 1024 rows; host rotates yi so the self-match diagonal sits in
  the local diagonal block.  Per 128-row tile and per 512-column chunk:
    ps_t = yit_loc @ yinT       -> db_c = sqrt(...)*2^-16   (ACT evict, f32)
    ps_a = yin_loc @ yinT       -> s_q  = fp16(s)           (ACT/DVE evict)
    ps_c = I_fp16 @ s_q + I_f32r @ db_c   (identity matmuls, f32 PSUM)
         = s_q + db*2^-16
  The DVE does ONE max8 scan per chunk over ps_c.  Because s_q is an exact
  fp16 value and db*2^-16 < ulp_fp16(s_q)/2 for any candidate-sized s, an
  fp16 round-trip of a candidate value recovers s_q exactly and the residual
  times 2^16 recovers db to ~2^-9 — enough for the hinge (validated offline:
  rel err ~1.2e-3 vs reference on the fixed dataset).  The self column is
  rank0 of its chunk (s=1 dominates); dis(yin_i,yit_i) for e2 comes from the
  diagonal block of db_c.  Top-16 threshold/mask + hinge run on [128, 128]
  candidate tiles.  No full-row elementwise work exists anywhere: per chunk
  the full-width ops are 4 matmuls (PE), one sqrt eviction (ACT), one fp16
  copy (ACT/DVE split) and one max8 scan (DVE).
"""

import os
import numpy as np

N, D = 8192, 128
NCORES = 8
ROWS = N // NCORES          # 1024 rows per core
NRT = ROWS // 128           # 8 row-tiles per core
NT = N // 128               # 64 column tiles
CH = 512                    # PSUM chunk width (1 bank)
NCH = N // CH               # 16 chunks per row-tile
T_THR = 0.0025
MARGIN = 0.5
EPS = 1e-12
C0 = 0.5 + 0.25e-12         # dis = sqrt(s*(-0.5) + C0)
PK = 65536.0                # db packing scale: C = s_q + db/PK
NEG = -1.0e30               # match_replace fill
KNOCK = -60000.0            # fp16-safe knock value

_CACHE = {}


def _build_module():
    import concourse.bass as bass  # noqa: F401
    import concourse.tile as tile
    from contextlib import ExitStack
    from concourse import bacc, mybir

    CFG = os.environ.get("BLCD_CFG", "")
    def knob(name, default):
        for part in CFG.split(","):
            if part.startswith(name + "="):
                return int(part.split("=")[1])
        return default
    SQD = knob("sqd", 4)     # s_q-evict chunks PER 8 on DVE (rest ACT)
    HEVD = knob("hevd", 1)   # alternate head evictions onto DVE
    PST = knob("pst", 2)     # PSUM bufs for t (1024 wide)
    PSA = knob("psa", 2)     # PSUM bufs for s
    PSC = knob("psc", 2)     # PSUM bufs for packed C

    f32 = mybir.dt.float32
    f32r = mybir.dt.float32r
    fp16 = mybir.dt.float16
    AF = mybir.ActivationFunctionType
    ALU = mybir.AluOpType
    AX = mybir.AxisListType

    nc = bacc.Bacc("TRN2", target_bir_lowering=False, debug=False,
                   num_devices=NCORES)

    yi_d = nc.dram_tensor("yi_rot", [N, D], f32, kind="ExternalInput")
    yit_d = nc.dram_tensor("yit_loc", [ROWS, D], f32, kind="ExternalInput")
    eye_d = nc.dram_tensor("eye1", [128, 128], f32, kind="ExternalInput")
    out_d = nc.dram_tensor("out", [128, 2], f32, kind="ExternalOutput")

    yi_r = yi_d.ap().rearrange("(n p) d -> p n d", p=128)     # [128, 64, 128]
    yit_r = yit_d.ap().rearrange("(n p) d -> p n d", p=128)   # [128, 8, 128]

    with tile.TileContext(nc) as tc, ExitStack() as ctx:
        cpool = ctx.enter_context(tc.tile_pool(name="consts", bufs=1))
        ppool = ctx.enter_context(tc.tile_pool(name="persist", bufs=1))
        smpool = ctx.enter_context(tc.tile_pool(name="small", bufs=2))

        eye = cpool.tile([128, 128], f32)
        nc.sync.dma_start(eye[:], eye_d[:])
        eyer = cpool.tile([128, 128], f32r)
        nc.gpsimd.tensor_copy(eyer[:], eye[:])
        eyeh = cpool.tile([128, 128], fp16)
        nc.gpsimd.tensor_copy(eyeh[:], eye[:])
        c0b = cpool.tile([128, 1], f32)
        nc.gpsimd.memset(c0b[:], C0)
        c0s = cpool.tile([128, 1], f32)
        nc.gpsimd.memset(c0s[:], C0 / (PK * PK))
        epsb = cpool.tile([128, 1], f32)
        nc.gpsimd.memset(epsb[:], EPS)

        yinT = ppool.tile([128, N], f32r)        # normalized yi, transposed
        yitT = ppool.tile([128, ROWS], f32r)     # normalized yi_t, transposed
        e1acc = ppool.tile([128, NRT], f32)
        e2acc = ppool.tile([128, NRT], f32)

        # ---------------- head: normalize + transpose ----------------
        # processed in 8-block groups so early yinT columns unblock the
        # main-loop matmuls long before the whole head finishes
        with tc.tile_pool(name="headbig", bufs=4) as hbig, \
             tc.tile_pool(name="headsm", bufs=4) as hsm, \
             tc.tile_pool(name="headps", bufs=4, space="PSUM") as hpsum:
            order = [(yi_r, 0, yinT), (yit_r, 0, yitT)] + \
                    [(yi_r, g, yinT) for g in range(8, NT, 8)]
            for (src_r, g, dstT) in order:
                rows = hbig.tile([128, 8, 128], f32, tag="rows")
                nc.sync.dma_start(rows[:], src_r[:, g:g + 8, :])
                sqr = hbig.tile([128, 128], f32, tag="sqr")
                sq = hsm.tile([128, 8], f32, tag="sq")
                for jj in range(8):
                    # row norms via Square+accum: keeps the head off the DVE
                    nc.scalar.activation(sqr[:], rows[:, jj, :], AF.Square,
                                         accum_out=sq[:, jj:jj + 1])
                nrm = hsm.tile([128, 8], f32, tag="nrm")
                nc.scalar.activation(nrm[:], sq[:], AF.Sqrt, bias=epsb[:])
                rinv = hsm.tile([128, 8], f32, tag="rinv")
                nc.vector.reciprocal(rinv[:], nrm[:])
                for jj in range(8):
                    j = g + jj
                    # diag(rinv) built on Pool; PE matmul y.T @ diag(r)
                    # fuses the normalize scaling into the transpose
                    diagm = hsm.tile([128, 128], f32, tag="diagm")
                    nc.gpsimd.tensor_scalar(diagm[:], eye[:],
                                            rinv[:, jj:jj + 1], None,
                                            ALU.mult)
                    ps = hpsum.tile([128, 128], f32, tag="tps")
                    nc.tensor.matmul(ps[:], rows[:, jj, :], diagm[:],
                                     start=True, stop=True)
                    if HEVD and j % 2 == 1:
                        nc.vector.tensor_copy(dstT[:, j * 128:(j + 1) * 128],
                                              ps[:])
                    else:
                        nc.scalar.copy(dstT[:, j * 128:(j + 1) * 128], ps[:])

        # ---------------- main loop over 8 row-tiles ----------------
        with tc.tile_pool(name="dbp", bufs=3) as dbpool, \
             tc.tile_pool(name="sqp", bufs=3) as sqpool, \
             tc.tile_pool(name="ps_t", bufs=PST, space="PSUM") as ps_tpool, \
             tc.tile_pool(name="ps_a", bufs=PSA, space="PSUM") as ps_apool, \
             tc.tile_pool(name="ps_c", bufs=PSC, space="PSUM") as ps_cpool:
            for rt in range(NRT):
                lhs_s = yinT[:, rt * 128:(rt + 1) * 128]
                lhs_t = yitT[:, rt * 128:(rt + 1) * 128]
                cand = smpool.tile([128, NCH * 8], f32, tag="cand")
                dis_td = smpool.tile([128, 1], f32, tag="dtd")
                c_self = (rt * 128) // CH
                o_self = (rt * 128) % CH
                for cp in range(NCH // 2):
                    ps_t = ps_tpool.tile([128, 2 * CH], f32, tag="pt")
                    db_c2 = dbpool.tile([128, 2 * CH], f32r, tag="db")
                    for h in range(2):
                        cc = 2 * cp + h
                        rhs = yinT[:, cc * CH:(cc + 1) * CH]
                        nc.tensor.matmul(ps_t[:, h * CH:(h + 1) * CH],
                                         lhs_t, rhs, start=True, stop=True)
                    # db pre-scaled by 1/PK: sqrt((-0.5 t + C0)/PK^2)
                    nc.scalar.activation(db_c2[:], ps_t[:], AF.Sqrt,
                                         scale=-0.5 / (PK * PK), bias=c0s[:])
                    if cp == c_self // 2:
                        # e2: dis(yin_i, yit_i)/PK on the diagonal block
                        doff = (c_self % 2) * CH + o_self
                        dsl = slice(doff, doff + 128)
                        tdscr = smpool.tile([128, 128], f32, tag="tdscr")
                        nc.gpsimd.tensor_tensor(tdscr[:], db_c2[:, dsl],
                                                eye[:], op=ALU.mult)
                        tdr = smpool.tile([128, 1], f32, tag="tdr")
                        nc.vector.tensor_reduce(tdr[:], tdscr[:], op=ALU.add,
                                                axis=AX.X)
                        nc.gpsimd.tensor_scalar(dis_td[:], tdr[:], PK, None,
                                                ALU.mult)
                    for h in range(2):
                        cc = 2 * cp + h
                        rhs = yinT[:, cc * CH:(cc + 1) * CH]
                        ps_a = ps_apool.tile([128, CH], f32)
                        nc.tensor.matmul(ps_a[:], lhs_s, rhs, start=True,
                                         stop=True)
                        s_q = sqpool.tile([128, CH], fp16, tag="sq")
                        if cc % 8 >= 8 - SQD:
                            nc.vector.tensor_copy(s_q[:], ps_a[:])
                        else:
                            nc.scalar.copy(s_q[:], ps_a[:])
                        ps_c = ps_cpool.tile([128, CH], f32)
                        nc.tensor.matmul(ps_c[:], eyeh[:], s_q[:],
                                         start=True, stop=False)
                        nc.tensor.matmul(ps_c[:], eyer[:],
                                         db_c2[:, h * CH:(h + 1) * CH],
                                         start=False, stop=True)
                        nc.vector.max(cand[:, cc * 8:(cc + 1) * 8], ps_c[:])

                # ---- candidate-space math ([128, 128] tiles) ----
                # knock self (rank0 of its chunk; s=1 dominates)
                nc.gpsimd.memset(cand[:, c_self * 8:c_self * 8 + 1], KNOCK)

                # theta = 16th largest candidate
                r1 = smpool.tile([128, 8], f32, tag="r1")
                r2 = smpool.tile([128, 8], f32, tag="r2")
                cbk = smpool.tile([128, NCH * 8], f32, tag="cbk")
                nc.vector.max(r1[:], cand[:])
                nc.vector.match_replace(cbk[:], r1[:], cand[:], NEG)
                nc.vector.max(r2[:], cbk[:])

                # decode: s_q = fp16 round-trip, db = (C - s_q)*PK
                cq = smpool.tile([128, NCH * 8], fp16, tag="cq")
                nc.scalar.copy(cq[:], cand[:])
                d64 = smpool.tile([128, NCH * 8], f32, tag="d64")
                nc.gpsimd.tensor_tensor(d64[:], cand[:], cq[:],
                                        op=ALU.subtract)
                db64 = smpool.tile([128, NCH * 8], f32, tag="db64")
                nc.gpsimd.tensor_scalar(db64[:], d64[:], PK, None,
                                        ALU.mult)
                da64 = smpool.tile([128, NCH * 8], f32, tag="da64")
                nc.scalar.activation(da64[:], cq[:], AF.Sqrt,
                                     scale=-0.5, bias=c0b[:])
                mk64 = smpool.tile([128, NCH * 8], f32, tag="mk64")
                nc.vector.tensor_scalar(mk64[:], cand[:], r2[:, 7:8], None,
                                        ALU.is_ge)
                w = smpool.tile([128, NCH * 8], f32, tag="w")
                nc.vector.tensor_tensor(w[:], da64[:], db64[:],
                                        op=ALU.subtract)
                nc.vector.tensor_tensor(w[:], w[:], mk64[:], op=ALU.mult)
                nc.vector.tensor_tensor(w[:], w[:], w[:], op=ALU.mult)
                # accum_out reduces with op1 -> relu (max) and summing
                # accumulate (add) stay separate instructions
                nc.vector.tensor_scalar(w[:], w[:], T_THR, 0.0,
                                        ALU.subtract, ALU.max)
                nc.vector.tensor_scalar(w[:], w[:], 1.0, None,
                                        ALU.mult, ALU.add,
                                        accum_out=e1acc[:, rt:rt + 1])

                # e2 row terms: nearest neighbor (rank-1 after self-knock)
                r1a = smpool.tile([128, 8], f32, tag="r1a")
                nc.vector.max(r1a[:], cand[:])
                r1aq = smpool.tile([128, 1], fp16, tag="r1aq")
                nc.scalar.copy(r1aq[:], r1a[:, 0:1])
                dis_nn = smpool.tile([128, 1], f32, tag="dnn")
                nc.scalar.activation(dis_nn[:], r1aq[:], AF.Sqrt,
                                     scale=-0.5, bias=c0b[:])
                o2 = smpool.tile([128, 1], f32, tag="o2")
                nc.vector.tensor_scalar(o2[:], dis_td[:], dis_nn[:, 0:1],
                                        MARGIN, ALU.subtract, ALU.add)
                nc.vector.tensor_scalar(e2acc[:, rt:rt + 1], o2[:], 0.0, None,
                                        ALU.max)

        # ---------------- tail: reduce + store ----------------
        e1r = smpool.tile([128, 1], f32, tag="e1r")
        e2r = smpool.tile([128, 1], f32, tag="e2r")
        nc.vector.tensor_reduce(e1r[:], e1acc[:], op=ALU.add, axis=AX.X)
        nc.vector.tensor_reduce(e2r[:], e2acc[:], op=ALU.add, axis=AX.X)
        nc.sync.dma_start(out_d[:, 0:1], e1r[:])
        nc.sync.dma_start(out_d[:, 1:2], e2r[:])

    nc.compile()
    return nc


def kernel(yi: np.ndarray, yi_t: np.ndarray):
    from concourse.bass_utils import run_bass_kernel_spmd

    if "nc" not in _CACHE:
        _CACHE["nc"] = _build_module()
    nc = _CACHE["nc"]

    yi = np.ascontiguousarray(np.asarray(yi, dtype=np.float32))
    yi_t = np.ascontiguousarray(np.asarray(yi_t, dtype=np.float32))
    eye1 = np.eye(128, dtype=np.float32)

    in_maps = []
    for c in range(NCORES):
        lo = c * ROWS
        yi_rot = np.concatenate([yi[lo:], yi[:lo]], axis=0)
        in_maps.append({
            "yi_rot": np.ascontiguousarray(yi_rot),
            "yit_loc": np.ascontiguousarray(yi_t[lo:lo + ROWS]),
            "eye1": eye1,
        })

    res = run_bass_kernel_spmd(nc, in_maps, list(range(NCORES))).results

    e1 = np.float64(0.0)
    e2 = np.float64(0.0)
    for c in range(NCORES):
        out = res[c]["out"]
        e1 += out[:, 0].astype(np.float64).sum()
        e2 += out[:, 1].astype(np.float64).sum()
    e1 = np.float32(e1)
    e2 = np.float32(e2)
    return (np.float32(e1 + e2), e1, e2)


# revision 21
# speedup vs baseline: 1.1877x; 1.0577x over previous
"""Trainium2 Bass kernel for nn_BLCD_Loss (retrieval kNN hinge loss).

Math (reference):
  yin = l2norm(yi), yit = l2norm(yi_t)
  dis[i,j] = sqrt(max(|yin_i|^2+|yin_j|^2-2 yin_i.yin_j, 0) + 1e-12)
  top-(K+1) smallest per row (rank0 = self); neighbors = ranks 1..16
  e1 = sum relu((0.5*sqrt(|yin_i-yin_j|^2+eps) - 0.5*sqrt(|yit_i-yin_j|^2+eps))^2 - T)
  e2 = sum relu(0.5*sqrt(|yin_i-yit_i|^2+eps) + M - 0.5*sqrt(|yin_i-yij|^2+eps))

Kernel strategy (8 cores, SPMD) — single-scan packed-candidate scheme:
  Each core owns 1024 rows; host rotates yi so the self-match diagonal sits in
  the local diagonal block.  Per 128-row tile and per 512-column chunk:
    ps_t = yit_loc @ yinT       -> db_c = sqrt(...)*2^-16   (ACT evict, f32)
    ps_a = yin_loc @ yinT       -> s_q  = fp16(s)           (ACT/DVE evict)
    ps_c = I_fp16 @ s_q + I_f32r @ db_c   (identity matmuls, f32 PSUM)
         = s_q + db*2^-16
  The DVE does ONE max8 scan per chunk over ps_c.  Because s_q is an exact
  fp16 value and db*2^-16 < ulp_fp16(s_q)/2 for any candidate-sized s, an
  fp16 round-trip of a candidate value recovers s_q exactly and the residual
  times 2^16 recovers db to ~2^-9 — enough for the hinge (validated offline:
  rel err ~1.2e-3 vs reference on the fixed dataset).  The self column is
  rank0 of its chunk (s=1 dominates); dis(yin_i,yit_i) for e2 comes from the
  diagonal block of db_c.  Top-16 threshold/mask + hinge run on [128, 128]
  candidate tiles.  No full-row elementwise work exists anywhere: per chunk
  the full-width ops are 4 matmuls (PE), one sqrt eviction (ACT), one fp16
  copy (ACT/DVE split) and one max8 scan (DVE).
"""

import os
import numpy as np

N, D = 8192, 128
NCORES = 8
ROWS = N // NCORES          # 1024 rows per core
NRT = ROWS // 128           # 8 row-tiles per core
NT = N // 128               # 64 column tiles
CH = 512                    # PSUM chunk width (1 bank)
NCH = N // CH               # 16 chunks per row-tile
T_THR = 0.0025
MARGIN = 0.5
EPS = 1e-12
C0 = 0.5 + 0.25e-12         # dis = sqrt(s*(-0.5) + C0)
PK = 65536.0                # db packing scale: C = s_q + db/PK
NEG = -1.0e30               # match_replace fill
KNOCK = -60000.0            # fp16-safe knock value

_CACHE = {}


def _build_module():
    import concourse.bass as bass  # noqa: F401
    import concourse.tile as tile
    from contextlib import ExitStack
    from concourse import bacc, mybir

    CFG = os.environ.get("BLCD_CFG", "")
    def knob(name, default):
        for part in CFG.split(","):
            if part.startswith(name + "="):
                return int(part.split("=")[1])
        return default
    SQD = knob("sqd", 2)     # s_q-evict chunks PER 8 on DVE (rest ACT)
    HEVD = knob("hevd", 1)   # alternate head evictions onto DVE
    PST = knob("pst", 2)     # PSUM bufs for t (1024 wide)
    PSA = knob("psa", 2)     # PSUM bufs for s
    PSC = knob("psc", 2)     # PSUM bufs for packed C

    f32 = mybir.dt.float32
    f32r = mybir.dt.float32r
    fp16 = mybir.dt.float16
    AF = mybir.ActivationFunctionType
    ALU = mybir.AluOpType
    AX = mybir.AxisListType

    nc = bacc.Bacc("TRN2", target_bir_lowering=False, debug=False,
                   num_devices=NCORES)

    yi_d = nc.dram_tensor("yi_rot", [N, D], f32, kind="ExternalInput")
    yit_d = nc.dram_tensor("yit_loc", [ROWS, D], f32, kind="ExternalInput")
    eye_d = nc.dram_tensor("eye1", [128, 128], f32, kind="ExternalInput")
    out_d = nc.dram_tensor("out", [128, 2], f32, kind="ExternalOutput")

    yi_r = yi_d.ap().rearrange("(n p) d -> p n d", p=128)     # [128, 64, 128]
    yit_r = yit_d.ap().rearrange("(n p) d -> p n d", p=128)   # [128, 8, 128]

    with tile.TileContext(nc) as tc, ExitStack() as ctx:
        cpool = ctx.enter_context(tc.tile_pool(name="consts", bufs=1))
        ppool = ctx.enter_context(tc.tile_pool(name="persist", bufs=1))
        smpool = ctx.enter_context(tc.tile_pool(name="small", bufs=2))

        eye = cpool.tile([128, 128], f32)
        nc.sync.dma_start(eye[:], eye_d[:])
        eyer = cpool.tile([128, 128], f32r)
        nc.gpsimd.tensor_copy(eyer[:], eye[:])
        eyeh = cpool.tile([128, 128], fp16)
        nc.gpsimd.tensor_copy(eyeh[:], eye[:])
        c0b = cpool.tile([128, 1], f32)
        nc.gpsimd.memset(c0b[:], C0)
        c0s = cpool.tile([128, 1], f32)
        nc.gpsimd.memset(c0s[:], C0 / (PK * PK))
        epsb = cpool.tile([128, 1], f32)
        nc.gpsimd.memset(epsb[:], EPS)

        yinT = ppool.tile([128, N], f32r)        # normalized yi, transposed
        yitT = ppool.tile([128, ROWS], f32r)     # normalized yi_t, transposed
        e1acc = ppool.tile([128, NRT], f32)
        e2acc = ppool.tile([128, NRT], f32)

        # ---------------- head: normalize + transpose ----------------
        # processed in 8-block groups so early yinT columns unblock the
        # main-loop matmuls long before the whole head finishes
        with tc.tile_pool(name="headbig", bufs=4) as hbig, \
             tc.tile_pool(name="headsm", bufs=4) as hsm, \
             tc.tile_pool(name="headps", bufs=4, space="PSUM") as hpsum:
            order = [(yi_r, 0, yinT), (yit_r, 0, yitT)] + \
                    [(yi_r, g, yinT) for g in range(8, NT, 8)]
            for (src_r, g, dstT) in order:
                rows = hbig.tile([128, 8, 128], f32, tag="rows")
                nc.sync.dma_start(rows[:], src_r[:, g:g + 8, :])
                sqr = hbig.tile([128, 128], f32, tag="sqr")
                sq = hsm.tile([128, 8], f32, tag="sq")
                for jj in range(8):
                    # row norms via Square+accum: keeps the head off the DVE
                    nc.scalar.activation(sqr[:], rows[:, jj, :], AF.Square,
                                         accum_out=sq[:, jj:jj + 1])
                nrm = hsm.tile([128, 8], f32, tag="nrm")
                nc.scalar.activation(nrm[:], sq[:], AF.Sqrt, bias=epsb[:])
                rinv = hsm.tile([128, 8], f32, tag="rinv")
                nc.vector.reciprocal(rinv[:], nrm[:])
                for jj in range(8):
                    j = g + jj
                    # diag(rinv) built on Pool; PE matmul y.T @ diag(r)
                    # fuses the normalize scaling into the transpose
                    diagm = hsm.tile([128, 128], f32, tag="diagm")
                    nc.gpsimd.tensor_scalar(diagm[:], eye[:],
                                            rinv[:, jj:jj + 1], None,
                                            ALU.mult)
                    ps = hpsum.tile([128, 128], f32, tag="tps")
                    nc.tensor.matmul(ps[:], rows[:, jj, :], diagm[:],
                                     start=True, stop=True)
                    if HEVD and j % 2 == 1:
                        nc.vector.tensor_copy(dstT[:, j * 128:(j + 1) * 128],
                                              ps[:])
                    else:
                        nc.scalar.copy(dstT[:, j * 128:(j + 1) * 128], ps[:])

        # ---------------- main loop over 8 row-tiles ----------------
        with tc.tile_pool(name="dbp", bufs=3) as dbpool, \
             tc.tile_pool(name="sqp", bufs=3) as sqpool, \
             tc.tile_pool(name="ps_t", bufs=PST, space="PSUM") as ps_tpool, \
             tc.tile_pool(name="ps_a", bufs=PSA, space="PSUM") as ps_apool, \
             tc.tile_pool(name="ps_c", bufs=PSC, space="PSUM") as ps_cpool:
            for rt in range(NRT):
                lhs_s = yinT[:, rt * 128:(rt + 1) * 128]
                lhs_t = yitT[:, rt * 128:(rt + 1) * 128]
                cand = smpool.tile([128, NCH * 8], f32, tag="cand")
                dis_td = smpool.tile([128, 1], f32, tag="dtd")
                c_self = (rt * 128) // CH
                o_self = (rt * 128) % CH
                for cp in range(NCH // 2):
                    ps_t = ps_tpool.tile([128, 2 * CH], f32, tag="pt")
                    db_c2 = dbpool.tile([128, 2 * CH], f32r, tag="db")
                    for h in range(2):
                        cc = 2 * cp + h
                        rhs = yinT[:, cc * CH:(cc + 1) * CH]
                        nc.tensor.matmul(ps_t[:, h * CH:(h + 1) * CH],
                                         lhs_t, rhs, start=True, stop=True)
                    # db pre-scaled by 1/PK: sqrt((-0.5 t + C0)/PK^2)
                    nc.scalar.activation(db_c2[:], ps_t[:], AF.Sqrt,
                                         scale=-0.5 / (PK * PK), bias=c0s[:])
                    if cp == c_self // 2:
                        # e2: dis(yin_i, yit_i)/PK on the diagonal block
                        doff = (c_self % 2) * CH + o_self
                        dsl = slice(doff, doff + 128)
                        tdscr = smpool.tile([128, 128], f32, tag="tdscr")
                        nc.gpsimd.tensor_tensor(tdscr[:], db_c2[:, dsl],
                                                eye[:], op=ALU.mult)
                        tdr = smpool.tile([128, 1], f32, tag="tdr")
                        nc.vector.tensor_reduce(tdr[:], tdscr[:], op=ALU.add,
                                                axis=AX.X)
                        nc.gpsimd.tensor_scalar(dis_td[:], tdr[:], PK, None,
                                                ALU.mult)
                    for h in range(2):
                        cc = 2 * cp + h
                        rhs = yinT[:, cc * CH:(cc + 1) * CH]
                        ps_a = ps_apool.tile([128, CH], f32)
                        nc.tensor.matmul(ps_a[:], lhs_s, rhs, start=True,
                                         stop=True)
                        s_q = sqpool.tile([128, CH], fp16, tag="sq")
                        if cc % 8 >= 8 - SQD:
                            nc.vector.tensor_copy(s_q[:], ps_a[:])
                        else:
                            nc.scalar.copy(s_q[:], ps_a[:])
                        ps_c = ps_cpool.tile([128, CH], f32)
                        nc.tensor.matmul(ps_c[:], eyeh[:], s_q[:],
                                         start=True, stop=False)
                        nc.tensor.matmul(ps_c[:], eyer[:],
                                         db_c2[:, h * CH:(h + 1) * CH],
                                         start=False, stop=True)
                        nc.vector.max(cand[:, cc * 8:(cc + 1) * 8], ps_c[:])

                # ---- candidate-space math ([128, 128] tiles) ----
                # knock self (rank0 of its chunk; s=1 dominates)
                nc.gpsimd.memset(cand[:, c_self * 8:c_self * 8 + 1], KNOCK)

                # theta = 16th largest candidate
                r1 = smpool.tile([128, 8], f32, tag="r1")
                r2 = smpool.tile([128, 8], f32, tag="r2")
                cbk = smpool.tile([128, NCH * 8], f32, tag="cbk")
                nc.vector.max(r1[:], cand[:])
                nc.vector.match_replace(cbk[:], r1[:], cand[:], NEG)
                nc.vector.max(r2[:], cbk[:])

                # decode: s_q = fp16 round-trip, db = (C - s_q)*PK
                cq = smpool.tile([128, NCH * 8], fp16, tag="cq")
                nc.scalar.copy(cq[:], cand[:])
                d64 = smpool.tile([128, NCH * 8], f32, tag="d64")
                nc.gpsimd.tensor_tensor(d64[:], cand[:], cq[:],
                                        op=ALU.subtract)
                db64 = smpool.tile([128, NCH * 8], f32, tag="db64")
                nc.gpsimd.tensor_scalar(db64[:], d64[:], PK, None,
                                        ALU.mult)
                da64 = smpool.tile([128, NCH * 8], f32, tag="da64")
                nc.scalar.activation(da64[:], cq[:], AF.Sqrt,
                                     scale=-0.5, bias=c0b[:])
                mk64 = smpool.tile([128, NCH * 8], f32, tag="mk64")
                nc.gpsimd.tensor_scalar(mk64[:], cand[:], r2[:, 7:8], None,
                                        ALU.is_ge)
                w = smpool.tile([128, NCH * 8], f32, tag="w")
                nc.gpsimd.tensor_tensor(w[:], da64[:], db64[:],
                                        op=ALU.subtract)
                nc.gpsimd.tensor_tensor(w[:], w[:], mk64[:], op=ALU.mult)
                nc.gpsimd.tensor_tensor(w[:], w[:], w[:], op=ALU.mult)
                # accum_out reduces with op1 -> relu (max) and summing
                # accumulate (add) stay separate instructions
                nc.gpsimd.tensor_scalar(w[:], w[:], T_THR, 0.0,
                                        ALU.subtract, ALU.max)
                nc.vector.tensor_scalar(w[:], w[:], 1.0, None,
                                        ALU.mult, ALU.add,
                                        accum_out=e1acc[:, rt:rt + 1])

                # e2 row terms: nearest neighbor (rank-1 after self-knock)
                r1a = smpool.tile([128, 8], f32, tag="r1a")
                nc.vector.max(r1a[:], cand[:])
                r1aq = smpool.tile([128, 1], fp16, tag="r1aq")
                nc.scalar.copy(r1aq[:], r1a[:, 0:1])
                dis_nn = smpool.tile([128, 1], f32, tag="dnn")
                nc.scalar.activation(dis_nn[:], r1aq[:], AF.Sqrt,
                                     scale=-0.5, bias=c0b[:])
                o2 = smpool.tile([128, 1], f32, tag="o2")
                nc.vector.tensor_scalar(o2[:], dis_td[:], dis_nn[:, 0:1],
                                        MARGIN, ALU.subtract, ALU.add)
                nc.vector.tensor_scalar(e2acc[:, rt:rt + 1], o2[:], 0.0, None,
                                        ALU.max)

        # ---------------- tail: reduce + store ----------------
        e1r = smpool.tile([128, 1], f32, tag="e1r")
        e2r = smpool.tile([128, 1], f32, tag="e2r")
        nc.vector.tensor_reduce(e1r[:], e1acc[:], op=ALU.add, axis=AX.X)
        nc.vector.tensor_reduce(e2r[:], e2acc[:], op=ALU.add, axis=AX.X)
        nc.sync.dma_start(out_d[:, 0:1], e1r[:])
        nc.sync.dma_start(out_d[:, 1:2], e2r[:])

    nc.compile()
    return nc


def kernel(yi: np.ndarray, yi_t: np.ndarray):
    from concourse.bass_utils import run_bass_kernel_spmd

    if "nc" not in _CACHE:
        _CACHE["nc"] = _build_module()
    nc = _CACHE["nc"]

    yi = np.ascontiguousarray(np.asarray(yi, dtype=np.float32))
    yi_t = np.ascontiguousarray(np.asarray(yi_t, dtype=np.float32))
    eye1 = np.eye(128, dtype=np.float32)

    in_maps = []
    for c in range(NCORES):
        lo = c * ROWS
        yi_rot = np.concatenate([yi[lo:], yi[:lo]], axis=0)
        in_maps.append({
            "yi_rot": np.ascontiguousarray(yi_rot),
            "yit_loc": np.ascontiguousarray(yi_t[lo:lo + ROWS]),
            "eye1": eye1,
        })

    res = run_bass_kernel_spmd(nc, in_maps, list(range(NCORES))).results

    e1 = np.float64(0.0)
    e2 = np.float64(0.0)
    for c in range(NCORES):
        out = res[c]["out"]
        e1 += out[:, 0].astype(np.float64).sum()
        e2 += out[:, 1].astype(np.float64).sum()
    e1 = np.float32(e1)
    e2 = np.float32(e2)
    return (np.float32(e1 + e2), e1, e2)
